# revision 1
# baseline (speedup 1.0000x reference)
"""Multi-head attention (B=2, S=2048, D=1024, H=16) on 8 TRN2 NeuronCores.

Sharding: tensor-parallel over heads x data-parallel over batch.
Core c handles batch b = c//4 and heads 4*(c%4) .. 4*(c%4)+3 (256 of the
1024 projected dims).  wq/wk/wv are split column-wise (rows of the [out,in]
weight), wo row-wise; each core emits a [S, D] partial of the output
projection and the host sums the 4 partials per batch.

Per-core kernel (all matmuls bf16, fp32 PSUM accumulation):
  1. Q^T, K^T [256, S] and V^T [256, S] projections (contraction over
     D=1024 in 8 chunks of 128; stationary = pre-transposed weight slices).
  2. V^T transposed back per 128-row chunk via PE transpose into a padded
     per-stack layout [v_h0 | 1 | 0..0 | v_h1] so each head's attnV matmul
     carries a ones-column that accumulates the softmax denominator.
  3. Per head: S^T[sk,sq] = K_h @ Q_h^T (K=64), P^T = exp(S^T/8) on ScalarE
     (PSUM->SBUF bf16), P^T *= mask^T (VectorE, bf16), O^T accumulated over
     sk chunks (K=128).  No max-subtraction: |scores/8| <~ 2.5 so exp is
     safely bounded, and masking multiplies by exact 0/1 after exp.
  4. ctx^T = O^T * (1/rowsum): bf16 reciprocal of the sums row, broadcast
     across partitions by a K=1 ones-row matmul, multiplied on VectorE.
  5. partial = ctx @ wo_slice^T via ctx^T-stationary matmuls, fp32 out; the
     stack-0 half is interleaved into heads 2/3, stack-1 DMA-accumulates.
"""

import sys

if "/opt/trn_rl_repo" not in sys.path:
    sys.path.insert(0, "/opt/trn_rl_repo")

import numpy as np
import ml_dtypes

B = 2
S = 2048
D = 1024
H = 16
DH = 64
P = 128
N_CORES = 8
HEADS_PER_CORE = 4
CORE_DIMS = HEADS_PER_CORE * DH  # 256
SQT = 512  # matmul moving free dim / PSUM bank
KC = D // P  # 8 contraction chunks for the input projections
BF16 = ml_dtypes.bfloat16

_CACHE = {}


def _build(s=S):
    """Build the single-core Bass program (same program on all 8 cores).

    Order: V projection (both stacks) -> V transposes -> Q -> K projections,
    then the four heads; the stack-0 output projection is interleaved into
    heads 2/3 as per-chunk fillers, the stack-1 half DMA-accumulates at the
    end.  Softmax normalization rides a ones-column through the attnV matmul
    (denominator), a bf16 reciprocal row, and a K=1 ones-matmul broadcast.
    """
    import concourse.bass as bass
    import concourse.bacc as bacc
    import concourse.mybir as mybir
    import concourse.tile as tile
    from concourse.masks import make_identity
    from contextlib import ExitStack

    dt = mybir.dt
    AF = mybir.ActivationFunctionType
    nsq = s // SQT  # Sq tiles
    nsk = s // P  # Sk chunks
    ntile = s // P
    sqg = [tuple(range(i, min(i + 2, nsq))) for i in range(0, nsq, 2)]
    VW = 208  # per-stack padded V row: [v_h0(64) | 1A | 1B | 0*62 | v_h1(64) | pad]
    # A-region lhsT (head hh=0): cols 0..64   = [v_h0 | ones]        M=65
    # B-region lhsT (head hh=1): cols 65..192 = [ones | 0*63 | v_h1] M=128

    nc = bacc.Bacc("TRN2", target_bir_lowering=False, debug=False)
    xqT = nc.declare_dram_parameter("xqT", [D, s], dt.bfloat16, isOutput=False)
    xkT = nc.declare_dram_parameter("xkT", [D, s], dt.bfloat16, isOutput=False)
    xvT = nc.declare_dram_parameter("xvT", [D, s], dt.bfloat16, isOutput=False)
    maskT = nc.declare_dram_parameter("maskT", [s, s], dt.bfloat16, isOutput=False)
    wqT = nc.declare_dram_parameter("wqT", [D, CORE_DIMS], dt.bfloat16, isOutput=False)
    wkT = nc.declare_dram_parameter("wkT", [D, CORE_DIMS], dt.bfloat16, isOutput=False)
    wvT = nc.declare_dram_parameter("wvT", [D, CORE_DIMS], dt.bfloat16, isOutput=False)
    woT = nc.declare_dram_parameter("woT", [CORE_DIMS, D], dt.bfloat16, isOutput=False)
    out = nc.declare_dram_parameter("out", [s, D], dt.float32, isOutput=True)

    with ExitStack() as ctx:
        tc = ctx.enter_context(tile.TileContext(nc))
        consts = ctx.enter_context(tc.tile_pool(name="consts", bufs=1))
        xpool = ctx.enter_context(tc.tile_pool(name="xpool", bufs=11))
        wpool = ctx.enter_context(tc.tile_pool(name="wpool", bufs=1))
        mpool = ctx.enter_context(tc.tile_pool(name="mpool", bufs=1))
        actpool = ctx.enter_context(tc.tile_pool(name="actpool", bufs=1))
        ptpool = ctx.enter_context(tc.tile_pool(name="ptpool", bufs=5))
        rpool = ctx.enter_context(tc.tile_pool(name="rpool", bufs=2))
        oqpool = ctx.enter_context(tc.tile_pool(name="oqpool", bufs=2))
        opool = ctx.enter_context(tc.tile_pool(name="opool", bufs=3))
        psA = ctx.enter_context(tc.tile_pool(name="psA", bufs=2, space="PSUM"))
        psB = ctx.enter_context(tc.tile_pool(name="psB", bufs=4, space="PSUM"))

        ident = consts.tile([P, P], dt.bfloat16)
        make_identity(nc, ident)
        onesb = consts.tile([P, P], dt.bfloat16)
        nc.vector.memset(onesb, 1.0)
        # warm the Exp activation table off the critical path
        warm = consts.tile([P, 1], dt.float32)
        nc.vector.memset(warm, 0.0)
        nc.scalar.activation(warm, warm, AF.Exp, scale=1.0)

        # --- resident weights ---
        wq_sb = wpool.tile([P, KC, CORE_DIMS], dt.bfloat16, tag="wq")
        wk_sb = wpool.tile([P, KC, CORE_DIMS], dt.bfloat16, tag="wk")
        wv_sb = wpool.tile([P, KC, CORE_DIMS], dt.bfloat16, tag="wv")
        wo_sb = wpool.tile([P, 2, D], dt.bfloat16, tag="wo")
        for wsb, wdr in ((wv_sb, wvT), (wq_sb, wqT), (wk_sb, wkT)):
            nc.gpsimd.dma_start(
                out=wsb, in_=wdr.rearrange("(kc p) m -> p kc m", p=P)
            )
        nc.gpsimd.dma_start(out=wo_sb, in_=woT.rearrange("(st p) n -> p st n", p=P))

        # --- projection outputs (transposed: [stack-dim 128, stack, s]) ---
        qT_sb = actpool.tile([P, 2, s], dt.bfloat16, tag="qT")
        kT_sb = actpool.tile([P, 2, s], dt.bfloat16, tag="kT")
        vT_sb = actpool.tile([P, 2, s], dt.bfloat16, tag="vT")
        ctxT_sb = actpool.tile([P, 2, s], dt.bfloat16, tag="ctxT")
        vpad = actpool.tile([P, nsk, 2, VW], dt.bfloat16, tag="vpad")

        # init vpad: zeros everywhere, ones at cols 64 (A) and 65 (B)
        nc.vector.memset(vpad, 0.0)
        nc.vector.memset(vpad[:, :, :, 64:66], 1.0)

        mask_sb = mpool.tile([P, nsk, s], dt.bfloat16, tag="mask")

        def load_x(xdr):
            xch = []
            for kc in range(KC):
                xt = xpool.tile([P, s], dt.bfloat16, name="x", tag="x")
                half = s // 2
                nc.gpsimd.dma_start(
                    out=xt[:, 0:half], in_=xdr[kc * P:(kc + 1) * P, 0:half]
                )
                nc.gpsimd.dma_start(
                    out=xt[:, half:s], in_=xdr[kc * P:(kc + 1) * P, half:s]
                )
                xch.append(xt)
            return xch

        def proj(xch, wsb, osb, st):
            pp = [None] * len(sqg)
            for kc in range(KC):
                lhsT = wsb[:, kc, st * P:(st + 1) * P]
                for gi, grp in enumerate(sqg):
                    if kc == 0:
                        pp[gi] = psA.tile(
                            [P, len(grp) * SQT], dt.float32, name="psA", tag="psA"
                        )
                    for j, sq in enumerate(grp):
                        nc.tensor.matmul(
                            pp[gi][:, j * SQT:(j + 1) * SQT],
                            lhsT,
                            xch[kc][:, sq * SQT:(sq + 1) * SQT],
                            start=(kc == 0),
                            stop=(kc == KC - 1),
                        )
            for gi, grp in enumerate(sqg):
                nc.vector.tensor_copy(
                    osb[:, st, grp[0] * SQT:(grp[-1] + 1) * SQT], pp[gi]
                )

        def all_proj():
            xch = load_x(xvT)
            for st in range(2):
                proj(xch, wv_sb, vT_sb, st)
            for st in range(2):
                for c in range(nsk):
                    pst = psB.tile([P, P], dt.bfloat16, name="pst", tag="psB")
                    nc.tensor.transpose(
                        pst, vT_sb[:, st, c * P:(c + 1) * P], ident
                    )
                    nc.scalar.copy(vpad[:, c, st, 0:64], pst[:, 0:64])
                    nc.scalar.copy(vpad[:, c, st, 129:193], pst[:, 64:128])
            xch = load_x(xqT)
            for st in range(2):
                proj(xch, wq_sb, qT_sb, st)
            xch = load_x(xkT)
            for c in range(nsk):
                nc.gpsimd.dma_start(
                    out=mask_sb[:, c, :], in_=maskT[c * P:(c + 1) * P, :]
                )
            for st in range(2):
                proj(xch, wk_sb, kT_sb, st)

        def outproj_stile(st, stile):
            accum = (
                mybir.AluOpType.bypass if st == 0 else mybir.AluOpType.add
            )
            pp = psA.tile([P, 2 * SQT], dt.float32, name="psA", tag="psA")
            lhsT = ctxT_sb[:, st, stile * P:(stile + 1) * P]
            for oh in range(2):
                nc.tensor.matmul(
                    pp[:, oh * SQT:(oh + 1) * SQT],
                    lhsT,
                    wo_sb[:, st, oh * SQT:(oh + 1) * SQT],
                    start=True,
                    stop=True,
                )
            ob = opool.tile([P, 2 * SQT], dt.float32, name="ob", tag="ob")
            if st == 0:
                nc.vector.tensor_copy(ob, pp)
            else:
                nc.scalar.copy(ob, pp)
            for oh in range(2):
                nc.gpsimd.dma_start(
                    out=out[stile * P:(stile + 1) * P, oh * SQT:(oh + 1) * SQT],
                    in_=ob[:, oh * SQT:(oh + 1) * SQT],
                    accum_op=accum,
                )

        def attention(h, fillers=()):
            st, hh = h // 2, h % 2
            hp = hh * 64  # partition base of this head inside the stack
            po = [
                psB.tile([P, SQT], dt.float32, name="psBo", tag="psB")
                for _ in range(nsq)
            ]
            fillers = list(fillers)
            for c in range(nsk):
                if fillers:
                    fillers.pop(0)()
                lhs_k = kT_sb[hp:hp + 64, st, c * P:(c + 1) * P]
                ps = [None] * len(sqg)
                for gi, grp in enumerate(sqg):
                    ps[gi] = psA.tile(
                        [P, len(grp) * SQT], dt.float32, name="psA", tag="psA"
                    )
                    for j, sq in enumerate(grp):
                        nc.tensor.matmul(
                            ps[gi][:, j * SQT:(j + 1) * SQT],
                            lhs_k,
                            qT_sb[hp:hp + 64, st, sq * SQT:(sq + 1) * SQT],
                            start=True,
                            stop=True,
                        )
                pt = ptpool.tile([P, s], dt.bfloat16, name="pt", tag="pt")
                for gi, grp in enumerate(sqg):
                    nc.scalar.activation(
                        pt[:, grp[0] * SQT:(grp[-1] + 1) * SQT], ps[gi], AF.Exp,
                        scale=0.125,
                    )
                nc.vector.tensor_mul(pt, pt, mask_sb[:, c, :])
                if hh == 0:
                    lhs_v = vpad[:, c, st, 0:65]  # [v|1] -> out parts 0..64
                    mrows = 65
                else:
                    lhs_v = vpad[:, c, st, 65:193]  # [1|0..|v] -> out parts 0..127
                    mrows = P
                for sq in range(nsq):
                    nc.tensor.matmul(
                        po[sq][0:mrows, :],
                        lhs_v,
                        pt[:, sq * SQT:(sq + 1) * SQT],
                        start=(c == 0),
                        stop=(c == nsk - 1),
                    )
            while fillers:
                fillers.pop(0)()
            # quick-release PSUM, then normalize: ctxT = O^T * (1/rowsum)
            srow = 64 if hh == 0 else 0  # partition holding the sums
            orow = 0 if hh == 0 else 64  # partition base of O^T rows
            mrows = 65 if hh == 0 else P
            for sq in range(nsq):
                oq = oqpool.tile([P, SQT], dt.float32, name="oq", tag="oq")
                nc.vector.tensor_copy(oq[0:mrows, :], po[sq][0:mrows, :])
                r = rpool.tile([P, SQT], dt.bfloat16, name="r", tag="r")
                with nc.allow_low_precision(reason="softmax denom bcast in bf16"):
                    nc.vector.reciprocal(r[srow:srow + 1, :], oq[srow:srow + 1, :])
                # broadcast 1/rowsum to all partitions via a K=1 ones matmul
                rb = psB.tile([P, SQT], dt.float32, name="psBr", tag="psB")
                nc.tensor.matmul(
                    rb,
                    onesb[srow:srow + 1, :],
                    r[srow:srow + 1, :],
                    start=True,
                    stop=True,
                )
                nc.vector.tensor_mul(
                    ctxT_sb[hp:hp + 64, st, sq * SQT:(sq + 1) * SQT],
                    oq[orow:orow + 64, :],
                    rb[orow:orow + 64, :],
                )

        all_proj()
        attention(0)
        attention(1)
        mk = lambda st, i: (lambda: outproj_stile(st, i))
        attention(2, fillers=[mk(0, i) for i in range(0, ntile // 2)])
        attention(3, fillers=[mk(0, i) for i in range(ntile // 2, ntile)])
        for i in range(ntile):
            outproj_stile(1, i)

    nc.compile()
    return nc



def _shard_inputs(query, key, value, mask, wq, wk, wv, wo):
    query = np.asarray(query, dtype=np.float32)
    key = np.asarray(key, dtype=np.float32)
    value = np.asarray(value, dtype=np.float32)
    mask = np.asarray(mask)
    wq = np.asarray(wq, dtype=np.float32)
    wk = np.asarray(wk, dtype=np.float32)
    wv = np.asarray(wv, dtype=np.float32)
    wo = np.asarray(wo, dtype=np.float32)

    xT = []
    mT = []
    for b in range(B):
        xT.append(
            tuple(
                np.ascontiguousarray(a[b].T).astype(BF16)
                for a in (query, key, value)
            )
        )
        mT.append(np.ascontiguousarray(mask[b].T).astype(BF16))

    in_maps = []
    for c in range(N_CORES):
        b, g = c // 4, c % 4
        hsel = slice(g * CORE_DIMS, (g + 1) * CORE_DIMS)
        in_maps.append(
            {
                "xqT": xT[b][0],
                "xkT": xT[b][1],
                "xvT": xT[b][2],
                "maskT": mT[b],
                "wqT": np.ascontiguousarray(wq[hsel].T).astype(BF16),
                "wkT": np.ascontiguousarray(wk[hsel].T).astype(BF16),
                "wvT": np.ascontiguousarray(wv[hsel].T).astype(BF16),
                "woT": np.ascontiguousarray(wo[:, hsel].T).astype(BF16),
            }
        )
    return in_maps


LAST_RESULTS = None  # BassKernelResults of the most recent kernel() call


def kernel(query, key, value, mask, wq, wk, wv, wo):
    global LAST_RESULTS
    from concourse import bass_utils

    if "nc" not in _CACHE:
        _CACHE["nc"] = _build()
    nc = _CACHE["nc"]

    in_maps = _shard_inputs(query, key, value, mask, wq, wk, wv, wo)
    res = bass_utils.run_bass_kernel_spmd(nc, in_maps, core_ids=list(range(N_CORES)))
    LAST_RESULTS = res

    outp = np.empty((B, S, D), dtype=np.float32)
    for b in range(B):
        acc = res.results[4 * b]["out"].astype(np.float32)
        for g in range(1, 4):
            acc = acc + res.results[4 * b + g]["out"]
        outp[b] = acc
    return outp



# revision 9
# speedup vs baseline: 1.2378x; 1.2378x over previous
"""Multi-head attention (B=2, S=2048, D=1024, H=16) on 8 TRN2 NeuronCores.

Sharding: tensor-parallel over heads x data-parallel over batch.
Core c handles batch b = c//4 and heads 4*(c%4) .. 4*(c%4)+3 (256 of the
1024 projected dims).  wq/wk/wv are split column-wise, wo row-wise; each
core emits a [S, D] bf16 partial of the output projection and the host
sums the 4 partials per batch.

Per-core kernel:
  1. Q^T/K^T projections in bf16 -> cast to fp8e4m3 in a *folded* layout
     [32-part-per-head, 2 dh-halves, S] so each head's QK^T runs as a
     single fp8 DoubleRow matmul (2 k-tiles of 32 = full dh=64 contraction,
     0.5 cyc/row).
  2. V projected directly to [sk, dims] layout (lhsT = x chunk), written
     into the padded per-stack vpad rows [v_h0 | 1 1 | .. | v_h1] whose
     ones-columns accumulate the softmax denominator through attnV.
  3. Attention pipeline over (sq-half, head, sk-chunk): scores (fp8 DR),
     exp on ScalarE (PSUM->SBUF bf16, scale=1/8, no max-subtraction),
     mask multiply on VectorE, attnV accumulation (bf16, M=65/128).
     attnV trails the scores stream by a software lag so the xv load and
     V projection (interleaved as fillers) stay off the critical path.
  4. Normalization: bf16 reciprocal of the denominator row, broadcast
     across partitions on GpSimd (partition_broadcast), one fused
     tensor_mul into ctxT.
  5. Output projection: both stacks accumulate in PSUM (no DMA-accum),
     one bf16 [128, D] store per sq-tile; first half interleaved into the
     second half's attention, remainder as the tail.
All DMAs issue from the SP (sync) sequencer in demand order.
"""

import sys

if "/opt/trn_rl_repo" not in sys.path:
    sys.path.insert(0, "/opt/trn_rl_repo")

import numpy as np
import ml_dtypes

B = 2
S = 2048
D = 1024
H = 16
DH = 64
P = 128
N_CORES = 8
HEADS_PER_CORE = 4
CORE_DIMS = HEADS_PER_CORE * DH  # 256
SQT = 512
HALF = 1024  # sq-half width
KC = D // P  # 8 contraction chunks for the input projections
BF16 = ml_dtypes.bfloat16

_CACHE = {}


def _build(s=S):
    import concourse.bass as bass
    import concourse.bacc as bacc
    import concourse.mybir as mybir
    import concourse.tile as tile
    from contextlib import ExitStack

    dt = mybir.dt
    AF = mybir.ActivationFunctionType
    DR = mybir.MatmulPerfMode.DoubleRow
    nsk = s // P  # 16 sk chunks
    nst = s // P  # 16 sq tiles (outproj)
    VW = 208  # vpad row: [v_h0(0:64) | 1 1 | 0*63 | v_h1(129:193) | pad]

    nc = bacc.Bacc("TRN2", target_bir_lowering=False, debug=False)
    xqT = nc.declare_dram_parameter("xqT", [D, s], dt.bfloat16, isOutput=False)
    xkT = nc.declare_dram_parameter("xkT", [D, s], dt.bfloat16, isOutput=False)
    xvT = nc.declare_dram_parameter("xvT", [D, s], dt.bfloat16, isOutput=False)
    maskT = nc.declare_dram_parameter("maskT", [s, s], dt.bfloat16, isOutput=False)
    wqT = nc.declare_dram_parameter("wqT", [D, CORE_DIMS], dt.bfloat16, isOutput=False)
    wkT = nc.declare_dram_parameter("wkT", [D, CORE_DIMS], dt.bfloat16, isOutput=False)
    wvT = nc.declare_dram_parameter("wvT", [D, CORE_DIMS], dt.bfloat16, isOutput=False)
    woT = nc.declare_dram_parameter("woT", [CORE_DIMS, D], dt.bfloat16, isOutput=False)
    out = nc.declare_dram_parameter("out", [s, D], dt.bfloat16, isOutput=True)

    with ExitStack() as ctx:
        tc = ctx.enter_context(tile.TileContext(nc))
        consts = ctx.enter_context(tc.tile_pool(name="consts", bufs=1))
        wpool = ctx.enter_context(tc.tile_pool(name="wpool", bufs=1))
        xpool = ctx.enter_context(tc.tile_pool(name="xpool", bufs=8))
        actpool = ctx.enter_context(tc.tile_pool(name="actpool", bufs=1))
        mpool = ctx.enter_context(tc.tile_pool(name="mpool", bufs=1))
        ptpool = ctx.enter_context(tc.tile_pool(name="ptpool", bufs=14))
        oqpool = ctx.enter_context(tc.tile_pool(name="oqpool", bufs=2))
        rpool = ctx.enter_context(tc.tile_pool(name="rpool", bufs=2))
        obpool = ctx.enter_context(tc.tile_pool(name="obpool", bufs=2))
        psA = ctx.enter_context(tc.tile_pool(name="psA", bufs=2, space="PSUM"))
        psPO = ctx.enter_context(tc.tile_pool(name="psPO", bufs=2, space="PSUM"))

        # warm the Exp activation table off the critical path
        warm = consts.tile([P, 1], dt.float32)
        nc.vector.memset(warm, 0.0)
        nc.scalar.activation(warm, warm, AF.Exp, scale=1.0)
        onesb = consts.tile([P, P], dt.bfloat16)
        nc.vector.memset(onesb, 1.0)

        # --- resident weights ---
        wq_sb = wpool.tile([P, KC, CORE_DIMS], dt.bfloat16, tag="wq")
        wk_sb = wpool.tile([P, KC, CORE_DIMS], dt.bfloat16, tag="wk")
        wv_sb = wpool.tile([P, KC, CORE_DIMS], dt.bfloat16, tag="wv")
        wo_sb = wpool.tile([P, 2, D], dt.bfloat16, tag="wo")

        # --- activations ---
        qT_sb = actpool.tile([P, 2, s], dt.float8e4, tag="qT")
        kT_sb = actpool.tile([P, 2, s], dt.float8e4, tag="kT")
        ctxT_sb = actpool.tile([P, 2, s], dt.bfloat16, tag="ctxT")
        vpad = actpool.tile([P, nsk, 2, VW], dt.bfloat16, tag="vpad")
        mask_sb = mpool.tile([P, nsk, s], dt.bfloat16, tag="mask")

        nc.vector.memset(vpad, 0.0)
        nc.vector.memset(vpad[:, :, :, 64:66], 1.0)

        # --- input DMAs, all on the SP sequencer in demand order ---
        def dma_w(wsb, wdr):
            nc.sync.dma_start(out=wsb, in_=wdr.rearrange("(kc p) m -> p kc m", p=P))

        def dma_x(xdr, kc):
            xt = xpool.tile([P, s], dt.bfloat16, name="x", tag="x")
            nc.sync.dma_start(out=xt, in_=xdr[kc * P:(kc + 1) * P, :])
            return xt

        def dma_m(c):
            nc.sync.dma_start(out=mask_sb[:, c, :], in_=maskT[c * P:(c + 1) * P, :])

        dma_w(wq_sb, wqT)
        xq = [dma_x(xqT, kc) for kc in range(KC)]
        dma_w(wk_sb, wkT)
        xk = [dma_x(xkT, kc) for kc in range(KC)]
        for c in range(8):
            dma_m(c)
        dma_w(wv_sb, wvT)
        xv = [dma_x(xvT, kc) for kc in range(KC)]
        dma_w(wo_sb, woT)
        for c in range(8, nsk):
            dma_m(c)

        # --- Q/K projections (bf16 matmul, fp8 folded output) ---
        def proj(xch, wsb, osb):
            for slot in range(2):
                pp = [
                    psA.tile([P, HALF], dt.float32, name="pp", tag="psA")
                    for _ in range(2)
                ]
                for kc in range(KC):
                    lhsT = wsb[:, kc, slot * P:(slot + 1) * P]
                    for grp in range(2):
                        for j in range(2):
                            col = grp * HALF + j * SQT
                            nc.tensor.matmul(
                                pp[grp][:, j * SQT:(j + 1) * SQT],
                                lhsT,
                                xch[kc][:, col:col + SQT],
                                start=(kc == 0),
                                stop=(kc == KC - 1),
                            )
                for grp in range(2):
                    with nc.allow_low_precision(reason="fp8 scores operands"):
                        nc.vector.tensor_copy(
                            osb[:, slot, grp * HALF:(grp + 1) * HALF], pp[grp]
                        )

        proj(xq, wq_sb, qT_sb)
        proj(xk, wk_sb, kT_sb)

        # --- pipeline work items ---
        pts = {}
        po = {}

        def scores_exp_mask(g):
            # DoubleRow with a stride-0 duplicated k-tile: computes 2x the
            # scores at 0.5 cyc/row; the doubling is folded into exp scale.
            Hh, h, c = g // 64, (g % 64) // 16, g % 16
            st, hh = h // 2, h % 2
            hp = hh * 64
            ps = psA.tile([P, HALF], dt.float32, name="ps", tag="psA")
            lhsT = kT_sb[hp:hp + 64, st:st + 1, c * P:(c + 1) * P].broadcast_to(
                [64, 2, P]
            )
            for i in range(2):
                col = Hh * HALF + i * SQT
                nc.tensor.matmul(
                    ps[:, i * SQT:(i + 1) * SQT],
                    lhsT,
                    qT_sb[hp:hp + 64, st:st + 1, col:col + SQT].broadcast_to(
                        [64, 2, SQT]
                    ),
                    start=True,
                    stop=True,
                    perf_mode=DR,
                )
            pt = ptpool.tile([P, HALF], dt.bfloat16, name="pt", tag="pt")
            nc.scalar.activation(pt, ps, AF.Exp, scale=0.0625)
            nc.vector.tensor_mul(pt, pt, mask_sb[:, c, Hh * HALF:(Hh + 1) * HALF])
            pts[(Hh, h, c)] = pt

        def vproj(c):
            pv = psA.tile([P, HALF], dt.float32, name="pv", tag="psA")
            for kc in range(KC):
                nc.tensor.matmul(
                    pv[:, 0:CORE_DIMS],
                    xv[kc][:, c * P:(c + 1) * P],
                    wv_sb[:, kc, :],
                    start=(kc == 0),
                    stop=(kc == KC - 1),
                )
            for st in range(2):
                nc.vector.tensor_copy(
                    vpad[:, c, st, 0:64], pv[:, st * P:st * P + 64]
                )
                nc.vector.tensor_copy(
                    vpad[:, c, st, 129:193], pv[:, st * P + 64:st * P + P]
                )

        def attnv(a):
            Hh, h, c = a // 64, (a % 64) // 16, a % 16
            st, hh = h // 2, h % 2
            if c == 0:
                po[(Hh, h)] = psPO.tile([P, HALF], dt.float32, name="po", tag="po")
            p = po[(Hh, h)]
            if hh == 0:
                lhs_v = vpad[:, c, st, 0:65]
                mrows = 65
            else:
                lhs_v = vpad[:, c, st, 65:193]
                mrows = P
            pt = pts.pop((Hh, h, c))
            for i in range(2):
                nc.tensor.matmul(
                    p[0:mrows, i * SQT:(i + 1) * SQT],
                    lhs_v,
                    pt[:, i * SQT:(i + 1) * SQT],
                    start=(c == 0),
                    stop=(c == nsk - 1),
                )

        def norm(Hh, h):
            st, hh = h // 2, h % 2
            hp = hh * 64
            srow = 64 if hh == 0 else 0
            orow = 0 if hh == 0 else 64
            mrows = 65 if hh == 0 else P
            p = po.pop((Hh, h))
            oq = oqpool.tile([P, HALF], dt.float32, name="oq", tag="oq")
            nc.vector.tensor_copy(oq[0:mrows, :], p[0:mrows, :])
            r = rpool.tile([P, HALF], dt.bfloat16, name="r", tag="r")
            with nc.allow_low_precision(reason="softmax denom bcast in bf16"):
                nc.vector.reciprocal(r[srow:srow + 1, :], oq[srow:srow + 1, :])
            # broadcast 1/rowsum to all partitions via a K=1 ones matmul
            rbb = psA.tile([P, HALF], dt.float32, name="rbb", tag="psA")
            for i in range(2):
                nc.tensor.matmul(
                    rbb[:, i * SQT:(i + 1) * SQT],
                    onesb[srow:srow + 1, :],
                    r[srow:srow + 1, i * SQT:(i + 1) * SQT],
                    start=True,
                    stop=True,
                )
            nc.vector.tensor_mul(
                ctxT_sb[hp:hp + 64, st, Hh * HALF:(Hh + 1) * HALF],
                oq[orow:orow + 64, :],
                rbb[orow:orow + 64, :],
            )

        obs = {}

        def outproj(stile):
            ob = obpool.tile([P, D], dt.bfloat16, name="ob", tag="ob")
            for dcol in range(2):
                pp = psA.tile([P, HALF], dt.float32, name="ppo", tag="psA")
                for st in range(2):
                    nc.tensor.matmul(
                        pp[:, 0:SQT],
                        ctxT_sb[:, st, stile * P:(stile + 1) * P],
                        wo_sb[:, st, dcol * SQT:(dcol + 1) * SQT],
                        start=(st == 0),
                        stop=(st == 1),
                    )
                if dcol == 0:
                    nc.scalar.copy(ob[:, dcol * SQT:(dcol + 1) * SQT], pp[:, 0:SQT])
                else:
                    nc.vector.tensor_copy(
                        ob[:, dcol * SQT:(dcol + 1) * SQT], pp[:, 0:SQT]
                    )
            nc.sync.dma_start(
                out=out[stile * P:(stile + 1) * P, :], in_=ob
            )

        # --- schedule: scores stream g=0..127; attnV trails by a lag ---
        NG = 128
        sched = {}

        def add(slot, fn):
            sched.setdefault(slot, []).append(fn)

        for a in range(NG):
            lag = 12 if a < 16 else 8
            Hh, h, c = a // 64, (a % 64) // 16, a % 16
            if a < 16:
                add(a + lag, (lambda c=c: vproj(c)))
            add(a + lag, (lambda a=a: attnv(a)))
            if c == 15:
                add(a + lag + 1, (lambda Hh=Hh, h=h: norm(Hh, h)))
        # outproj of sq-half 0 interleaved into half 1's stream
        for i, slot in enumerate(range(78, 126, 6)):
            add(slot, (lambda i=i: outproj(i)))

        max_slot = max(sched)
        for g in range(max_slot + 1):
            if g < NG:
                scores_exp_mask(g)
            for fn in sched.get(g, []):
                fn()
        # tail: remaining outproj tiles
        for i in range(8, nst):
            outproj(i)

    nc.compile()
    return nc


def _shard_inputs(query, key, value, mask, wq, wk, wv, wo):
    query = np.asarray(query, dtype=np.float32)
    key = np.asarray(key, dtype=np.float32)
    value = np.asarray(value, dtype=np.float32)
    mask = np.asarray(mask)
    wq = np.asarray(wq, dtype=np.float32)
    wk = np.asarray(wk, dtype=np.float32)
    wv = np.asarray(wv, dtype=np.float32)
    wo = np.asarray(wo, dtype=np.float32)

    xT = []
    mT = []
    for b in range(B):
        xT.append(
            tuple(
                np.ascontiguousarray(a[b].T).astype(BF16)
                for a in (query, key, value)
            )
        )
        mT.append(np.ascontiguousarray(mask[b].T).astype(BF16))

    in_maps = []
    for c in range(N_CORES):
        b, g = c // 4, c % 4
        hsel = slice(g * CORE_DIMS, (g + 1) * CORE_DIMS)
        in_maps.append(
            {
                "xqT": xT[b][0],
                "xkT": xT[b][1],
                "xvT": xT[b][2],
                "maskT": mT[b],
                "wqT": np.ascontiguousarray(wq[hsel].T).astype(BF16),
                "wkT": np.ascontiguousarray(wk[hsel].T).astype(BF16),
                "wvT": np.ascontiguousarray(wv[hsel].T).astype(BF16),
                "woT": np.ascontiguousarray(wo[:, hsel].T).astype(BF16),
            }
        )
    return in_maps


LAST_RESULTS = None  # BassKernelResults of the most recent kernel() call


def kernel(query, key, value, mask, wq, wk, wv, wo):
    global LAST_RESULTS
    from concourse import bass_utils

    if "nc" not in _CACHE:
        _CACHE["nc"] = _build()
    nc = _CACHE["nc"]

    in_maps = _shard_inputs(query, key, value, mask, wq, wk, wv, wo)
    res = bass_utils.run_bass_kernel_spmd(nc, in_maps, core_ids=list(range(N_CORES)))
    LAST_RESULTS = res

    outp = np.empty((B, S, D), dtype=np.float32)
    for b in range(B):
        acc = res.results[4 * b]["out"].astype(np.float32)
        for g in range(1, 4):
            acc = acc + res.results[4 * b + g]["out"].astype(np.float32)
        outp[b] = acc
    return outp


# revision 32
# speedup vs baseline: 1.3256x; 1.0709x over previous
"""Multi-head attention (B=2, S=2048, D=1024, H=16) on 8 TRN2 NeuronCores.

Sharding: tensor-parallel over heads x data-parallel over batch.
Core c handles batch b = c//4 and heads 4*(c%4) .. 4*(c%4)+3 (256 of the
1024 projected dims).  wq/wk/wv are split column-wise, wo row-wise; each
core emits a [S, D] bf16 partial of the output projection and the host
sums the 4 partials per batch.

Per-core kernel:
  1. Q^T/K^T projections in bf16; outputs cast to fp8e4m3.  Q is stored
     as a hi/lo fp8 split (q ~= q_hi + q_lo at ~bf16 precision).
  2. Scores for head h, sk-chunk c as ONE fp8 DoubleRow matmul: the
     stationary k8 tile is duplicated via a stride-0 AP, the moving pair
     is (q_hi, q_lo), so PSUM accumulates k8*(q_hi+q_lo) = k8*q at
     0.5 cycles/row.  exp on ScalarE (scale=1/8, no max subtraction),
     mask multiply on VectorE.
  3. V projected directly to [sk, dims] layout (lhsT = x chunk).
  4. attnV transposed: lhsT = P^T sq-chunk (M=128 full PE width), rhs =
     V columns (N=64) accumulating ctx^T' = [sq, dh] in 1-bank PSUM
     tiles; a parallel N=1 ones-matmul accumulates the softmax
     denominators per sq partition.
  5. Normalization per head: fp32 reciprocal of the denominator column,
     per-partition tensor_scalar multiply into a paired [sq, (hh, dh)]
     bf16 staging tile; an SBUF->SBUF DMA-transpose of each [128, 128]
     block restores the ctxT [dims, sq] layout for the output projection.
  6. Output projection accumulates both stacks in PSUM and stores one
     bf16 [128, D] tile per sq-chunk; the first half is interleaved into
     the second half's attention stream, the rest forms the tail.
All DMAs (loads, transposes, stores) issue from the SP sequencer.
"""

import sys

if "/opt/trn_rl_repo" not in sys.path:
    sys.path.insert(0, "/opt/trn_rl_repo")

import numpy as np
import ml_dtypes

B = 2
S = 2048
D = 1024
H = 16
DH = 64
P = 128
N_CORES = 8
HEADS_PER_CORE = 4
CORE_DIMS = HEADS_PER_CORE * DH  # 256
SQT = 512
HALF = 1024  # sq-half width
NSQC = HALF // P  # 8 sq chunks per half
KC = D // P  # 8 contraction chunks for the input projections
BF16 = ml_dtypes.bfloat16

_CACHE = {}


def _build(s=S):
    import concourse.bass as bass
    import concourse.bacc as bacc
    import concourse.mybir as mybir
    import concourse.tile as tile
    from contextlib import ExitStack

    dt = mybir.dt
    AF = mybir.ActivationFunctionType
    DR = mybir.MatmulPerfMode.DoubleRow
    nsk = s // P  # 16 sk chunks
    nst = s // P  # 16 sq tiles (outproj)

    nc = bacc.Bacc("TRN2", target_bir_lowering=False, debug=False)
    xqT = nc.declare_dram_parameter("xqT", [D, s], dt.bfloat16, isOutput=False)
    xkT = nc.declare_dram_parameter("xkT", [D, s], dt.bfloat16, isOutput=False)
    xvT = nc.declare_dram_parameter("xvT", [D, s], dt.bfloat16, isOutput=False)
    maskT = nc.declare_dram_parameter("maskT", [s, s], dt.bfloat16, isOutput=False)
    wqT = nc.declare_dram_parameter("wqT", [D, CORE_DIMS], dt.bfloat16, isOutput=False)
    wkT = nc.declare_dram_parameter("wkT", [D, CORE_DIMS], dt.bfloat16, isOutput=False)
    wvT = nc.declare_dram_parameter("wvT", [D, CORE_DIMS], dt.bfloat16, isOutput=False)
    woT = nc.declare_dram_parameter("woT", [CORE_DIMS, D], dt.bfloat16, isOutput=False)
    out = nc.declare_dram_parameter("out", [s, D], dt.bfloat16, isOutput=True)

    with ExitStack() as ctx:
        tc = ctx.enter_context(tile.TileContext(nc))
        consts = ctx.enter_context(tc.tile_pool(name="consts", bufs=1))
        wpool = ctx.enter_context(tc.tile_pool(name="wpool", bufs=1))
        xpool = ctx.enter_context(tc.tile_pool(name="xpool", bufs=8))
        actpool = ctx.enter_context(tc.tile_pool(name="actpool", bufs=1))
        mpool = ctx.enter_context(tc.tile_pool(name="mpool", bufs=1))
        ptpool = ctx.enter_context(tc.tile_pool(name="ptpool", bufs=22))
        rpool = ctx.enter_context(tc.tile_pool(name="rpool", bufs=2))
        ctpool = ctx.enter_context(tc.tile_pool(name="ctpool", bufs=2))
        obpool = ctx.enter_context(tc.tile_pool(name="obpool", bufs=4))
        psA = ctx.enter_context(tc.tile_pool(name="psA", bufs=2, space="PSUM"))
        psPO = ctx.enter_context(tc.tile_pool(name="psPO", bufs=2, space="PSUM"))

        # warm the Exp activation table off the critical path
        warm = consts.tile([P, 1], dt.float32)
        nc.vector.memset(warm, 0.0)
        nc.scalar.activation(warm, warm, AF.Exp, scale=1.0)
        onesb = consts.tile([P, 1], dt.bfloat16)
        nc.vector.memset(onesb, 1.0)

        # --- resident weights ---
        wq_sb = wpool.tile([P, KC, CORE_DIMS], dt.bfloat16, tag="wq")
        wk_sb = wpool.tile([P, KC, CORE_DIMS], dt.bfloat16, tag="wk")
        wv_sb = wpool.tile([P, KC, CORE_DIMS], dt.bfloat16, tag="wv")
        wo_sb = wpool.tile([P, 2, D], dt.bfloat16, tag="wo")

        # --- activations ---
        qT_sb = actpool.tile([P, 2, 2, s], dt.float8e4, tag="qT")  # (st, hi/lo)
        kT_sb = actpool.tile([P, 2, s], dt.float8e4, tag="kT")
        ctxT_sb = actpool.tile([P, 2, s], dt.bfloat16, tag="ctxT")
        # V in [sk, dims] layout: (chunk, stack, hh, dh)
        vpad = actpool.tile([P, nsk, 2, 2, DH], dt.bfloat16, tag="vpad")
        mask_sb = mpool.tile([P, nsk, s], dt.bfloat16, tag="mask")

        # --- input DMAs, all on the SP sequencer in demand order ---
        def dma_w(wsb, wdr):
            nc.sync.dma_start(out=wsb, in_=wdr.rearrange("(kc p) m -> p kc m", p=P))

        def dma_x(xdr, kc):
            xt = xpool.tile([P, s], dt.bfloat16, name="x", tag="x")
            nc.sync.dma_start(out=xt, in_=xdr[kc * P:(kc + 1) * P, :])
            return xt

        def dma_m(c):
            nc.sync.dma_start(out=mask_sb[:, c, :], in_=maskT[c * P:(c + 1) * P, :])

        dma_w(wq_sb, wqT)
        xq = [dma_x(xqT, kc) for kc in range(KC)]
        dma_w(wk_sb, wkT)
        xk = [dma_x(xkT, kc) for kc in range(KC)]
        dma_w(wv_sb, wvT)
        xv = [dma_x(xvT, kc) for kc in range(KC)]
        dma_w(wo_sb, woT)
        for c in range(nsk):
            dma_m(c)

        # --- Q/K projections ---
        # stack 0 through the psA ring up front (xq/xk DMA-paced); stack 1
        # in [P,512] quarters through the psPO ring, interleaved into the
        # first scores slots so the psA scores ring never blocks on them.
        def q_cast(pp, slot, col, w):
            # hi on ScalarE, lo = pp - hi on VectorE
            with nc.allow_low_precision(reason="fp8 hi/lo scores operands"):
                dst_hi = qT_sb[:, slot, 0, col:col + w]
                nc.scalar.copy(dst_hi, pp)
                nc.vector.tensor_sub(
                    qT_sb[:, slot, 1, col:col + w], pp, dst_hi
                )

        def k_cast(pp, slot, col, w, on_act):
            with nc.allow_low_precision(reason="fp8 scores operands"):
                if on_act:
                    nc.scalar.copy(kT_sb[:, slot, col:col + w], pp)
                else:
                    nc.vector.tensor_copy(kT_sb[:, slot, col:col + w], pp)

        def proj0(xch, wsb, cast):
            pp = [
                psA.tile([P, HALF], dt.float32, name="pp", tag="psA")
                for _ in range(2)
            ]
            for kc in range(KC):
                lhsT = wsb[:, kc, 0:P]
                for grp in range(2):
                    for j in range(2):
                        col = grp * HALF + j * SQT
                        nc.tensor.matmul(
                            pp[grp][:, j * SQT:(j + 1) * SQT],
                            lhsT,
                            xch[kc][:, col:col + SQT],
                            start=(kc == 0),
                            stop=(kc == KC - 1),
                        )
            for grp in range(2):
                cast(pp[grp], grp)

        proj0(xq, wq_sb, lambda pp, grp: q_cast(pp, 0, grp * HALF, HALF))
        proj0(xk, wk_sb, lambda pp, grp: k_cast(pp, 0, grp * HALF, HALF, grp == 0))

        def proj1_quarter(xch, wsb, q, cast):
            pp = psPO.tile([P, SQT], dt.float32, name="pp", tag="po")
            for kc in range(KC):
                nc.tensor.matmul(
                    pp,
                    wsb[:, kc, P:2 * P],
                    xch[kc][:, q * SQT:(q + 1) * SQT],
                    start=(kc == 0),
                    stop=(kc == KC - 1),
                )
            cast(pp, q)

        # --- pipeline state ---
        pts = {}
        po = {}
        cts = {}
        dens = {}

        def scores_exp_mask(g):
            Hh, h, c = g // 64, (g % 64) // 16, g % 16
            st, hh = h // 2, h % 2
            hp = hh * 64
            ps = psA.tile([P, HALF], dt.float32, name="ps", tag="psA")
            lhsT = kT_sb[hp:hp + 64, st:st + 1, c * P:(c + 1) * P].broadcast_to(
                [64, 2, P]
            )
            for i in range(2):
                col = Hh * HALF + i * SQT
                nc.tensor.matmul(
                    ps[:, i * SQT:(i + 1) * SQT],
                    lhsT,
                    qT_sb[hp:hp + 64, st, :, col:col + SQT],
                    start=True,
                    stop=True,
                    perf_mode=DR,
                )
            pt = ptpool.tile([P, HALF], dt.bfloat16, name="pt", tag="pt")
            nc.scalar.activation(pt, ps, AF.Exp, scale=0.125)
            nc.vector.tensor_mul(pt, pt, mask_sb[:, c, Hh * HALF:(Hh + 1) * HALF])
            pts[(Hh, h, c)] = pt

        def vproj(c):
            pv = psA.tile([P, HALF], dt.float32, name="pv", tag="psA")
            for kc in range(KC):
                nc.tensor.matmul(
                    pv[:, 0:CORE_DIMS],
                    xv[kc][:, c * P:(c + 1) * P],
                    wv_sb[:, kc, :],
                    start=(kc == 0),
                    stop=(kc == KC - 1),
                )
            for st in range(2):
                nc.vector.tensor_copy(
                    vpad[:, c, st, :, :], pv[:, st * P:(st + 1) * P]
                )

        def attnv(a):
            Hh, h, c = a // 64, (a % 64) // 16, a % 16
            st, hh = h // 2, h % 2
            if c == 0:
                po[(Hh, h)] = psPO.tile(
                    [P, NSQC, DH], dt.float32, name="po", tag="po"
                )
                dens[(Hh, h)] = psPO.tile(
                    [P, NSQC], dt.float32, name="den", tag="den", bufs=2
                )
            p = po[(Hh, h)]
            dn = dens[(Hh, h)]
            pt = pts.pop((Hh, h, c))
            for sqc in range(NSQC):
                lhsT = pt[:, sqc * P:(sqc + 1) * P]
                # one accumulation group per 2KB PSUM bank: start/stop only
                # on the first/last write of each tile
                nc.tensor.matmul(
                    p[:, sqc, :],
                    lhsT,
                    vpad[:, c, st, hh, :],
                    start=(c == 0 and sqc == 0),
                    stop=(c == nsk - 1 and sqc == NSQC - 1),
                )
                nc.tensor.matmul(
                    dn[:, sqc:sqc + 1],
                    lhsT,
                    onesb,
                    start=(c == 0 and sqc == 0),
                    stop=(c == nsk - 1 and sqc == NSQC - 1),
                )

        def norm(Hh, h):
            st, hh = h // 2, h % 2
            p = po.pop((Hh, h))
            if hh == 0:
                cts[(Hh, st)] = ctpool.tile(
                    [P, NSQC, 2, DH], dt.bfloat16, name="ct", tag="ct"
                )
            ct = cts[(Hh, st)]
            dn = dens.pop((Hh, h))
            rden = rpool.tile([P, NSQC], dt.float32, name="rden", tag="r")
            nc.vector.reciprocal(rden, dn)
            for sqc in range(NSQC):
                nc.vector.tensor_scalar_mul(
                    ct[:, sqc, hh, :], p[:, sqc, :], rden[:, sqc:sqc + 1]
                )
            if hh == 1:
                # restore [dims, sq] layout: one [128,128] DMA-transpose
                # per sq-chunk
                ct2 = cts.pop((Hh, st))
                for sqc in range(NSQC):
                    nc.sync.dma_start(
                        out=ctxT_sb[:, st, Hh * HALF + sqc * P:
                                    Hh * HALF + (sqc + 1) * P],
                        in_=ct2[:, sqc, :, :],
                        transpose=True,
                    )

        def outproj(stile, use_act=False):
            ob = obpool.tile([P, D], dt.bfloat16, name="ob", tag="ob")
            for dcol in range(2):
                pp = psA.tile([P, HALF], dt.float32, name="ppo", tag="psA")
                for st in range(2):
                    nc.tensor.matmul(
                        pp[:, 0:SQT],
                        ctxT_sb[:, st, stile * P:(stile + 1) * P],
                        wo_sb[:, st, dcol * SQT:(dcol + 1) * SQT],
                        start=(st == 0),
                        stop=(st == 1),
                    )
                if dcol == 0 and use_act:
                    nc.scalar.copy(ob[:, dcol * SQT:(dcol + 1) * SQT], pp[:, 0:SQT])
                else:
                    nc.vector.tensor_copy(
                        ob[:, dcol * SQT:(dcol + 1) * SQT], pp[:, 0:SQT]
                    )
            nc.sync.dma_start(out=out[stile * P:(stile + 1) * P, :], in_=ob)

        # --- schedule: scores stream g=0..127; attnV trails by a lag ---
        NG = 128
        sched = {}

        def add(slot, fn):
            sched.setdefault(slot, []).append(fn)

        # stack-1 projections occupy the first scores slots
        for q in range(4):
            add(q, (lambda q=q: proj1_quarter(
                xq, wq_sb, q, lambda pp, qq: q_cast(pp, 1, qq * SQT, SQT))))
        for q in range(4):
            add(4 + q, (lambda q=q: proj1_quarter(
                xk, wk_sb, q, lambda pp, qq: k_cast(pp, 1, qq * SQT, SQT, qq % 2 == 0))))

        for a in range(NG):
            Hh, h, c = a // 64, (a % 64) // 16, a % 16
            lag = (12 if a < 16 else 8) if Hh == 0 else 4
            if a < 16:
                add(a + lag, (lambda c=c: vproj(c)))
            add(a + lag, (lambda a=a: attnv(a)))
            if c == 15:
                add(a + lag + 1, (lambda Hh=Hh, h=h: norm(Hh, h)))
        # outproj of sq-half 0 interleaved into half 1's stream
        for i, slot in enumerate(range(80, 112, 4)):
            add(slot, (lambda i=i: outproj(i)))

        max_slot = max(sched)
        for g in range(max_slot + 1):
            if g < NG:
                scores_exp_mask(g)
            for fn in sched.get(g, []):
                fn()
        # tail: remaining outproj tiles
        for i in range(8, nst):
            outproj(i, use_act=True)

    nc.compile()
    return nc


def _shard_inputs(query, key, value, mask, wq, wk, wv, wo):
    query = np.asarray(query, dtype=np.float32)
    key = np.asarray(key, dtype=np.float32)
    value = np.asarray(value, dtype=np.float32)
    mask = np.asarray(mask)
    wq = np.asarray(wq, dtype=np.float32)
    wk = np.asarray(wk, dtype=np.float32)
    wv = np.asarray(wv, dtype=np.float32)
    wo = np.asarray(wo, dtype=np.float32)

    xT = []
    mT = []
    for b in range(B):
        xT.append(
            tuple(
                np.ascontiguousarray(a[b].T).astype(BF16)
                for a in (query, key, value)
            )
        )
        mT.append(np.ascontiguousarray(mask[b].T).astype(BF16))

    in_maps = []
    for c in range(N_CORES):
        b, g = c // 4, c % 4
        hsel = slice(g * CORE_DIMS, (g + 1) * CORE_DIMS)
        in_maps.append(
            {
                "xqT": xT[b][0],
                "xkT": xT[b][1],
                "xvT": xT[b][2],
                "maskT": mT[b],
                "wqT": np.ascontiguousarray(wq[hsel].T).astype(BF16),
                "wkT": np.ascontiguousarray(wk[hsel].T).astype(BF16),
                "wvT": np.ascontiguousarray(wv[hsel].T).astype(BF16),
                "woT": np.ascontiguousarray(wo[:, hsel].T).astype(BF16),
            }
        )
    return in_maps


LAST_RESULTS = None  # BassKernelResults of the most recent kernel() call


def kernel(query, key, value, mask, wq, wk, wv, wo):
    global LAST_RESULTS
    from concourse import bass_utils

    if "nc" not in _CACHE:
        _CACHE["nc"] = _build()
    nc = _CACHE["nc"]

    in_maps = _shard_inputs(query, key, value, mask, wq, wk, wv, wo)
    res = bass_utils.run_bass_kernel_spmd(nc, in_maps, core_ids=list(range(N_CORES)))
    LAST_RESULTS = res

    outp = np.empty((B, S, D), dtype=np.float32)
    for b in range(B):
        acc = res.results[4 * b]["out"].astype(np.float32)
        for g in range(1, 4):
            acc = acc + res.results[4 * b + g]["out"].astype(np.float32)
        outp[b] = acc
    return outp


# revision 39
# speedup vs baseline: 1.4506x; 1.0942x over previous
"""Multi-head attention (B=2, S=2048, D=1024, H=16) on 8 TRN2 NeuronCores.

Sharding: tensor-parallel over heads x data-parallel over batch.
Core c handles batch b = c//4 and heads 4*(c%4) .. 4*(c%4)+3 (256 of the
1024 projected dims).  wq/wk/wv are split column-wise, wo row-wise; each
core emits a [S, D] bf16 partial of the output projection and the host
sums the 4 partials per batch.

Per-core kernel:
  1. Q^T/K^T projections in bf16; outputs cast to fp8e4m3.  Q is stored
     as a hi/lo fp8 split (q ~= q_hi + q_lo at ~bf16 precision).
  2. Scores for head h, sk-chunk c as ONE fp8 DoubleRow matmul: the
     stationary k8 tile is duplicated via a stride-0 AP, the moving pair
     is (q_hi, q_lo), so PSUM accumulates k8*(q_hi+q_lo) = k8*q at
     0.5 cycles/row.  exp on ScalarE (scale=1/8, no max subtraction),
     mask multiply on VectorE.
  3. V projected directly to [sk, dims] layout (lhsT = x chunk).
  4. attnV transposed: lhsT = P^T sq-chunk (M=128 full PE width), rhs =
     V columns (N=64) accumulating ctx^T' = [sq, dh] in 1-bank PSUM
     tiles; a parallel N=1 ones-matmul accumulates the softmax
     denominators per sq partition.
  5. Normalization per head: fp32 reciprocal of the denominator column,
     per-partition tensor_scalar multiply into a paired [sq, (hh, dh)]
     bf16 staging tile; an SBUF->SBUF DMA-transpose of each [128, 128]
     block restores the ctxT [dims, sq] layout for the output projection.
  6. Output projection accumulates both stacks in PSUM and stores one
     bf16 [128, D] tile per sq-chunk; the first half is interleaved into
     the second half's attention stream, the rest forms the tail.
All DMAs (loads, transposes, stores) issue from the SP sequencer.
"""

import sys

if "/opt/trn_rl_repo" not in sys.path:
    sys.path.insert(0, "/opt/trn_rl_repo")

import numpy as np
import ml_dtypes

B = 2
S = 2048
D = 1024
H = 16
DH = 64
P = 128
N_CORES = 8
HEADS_PER_CORE = 4
CORE_DIMS = HEADS_PER_CORE * DH  # 256
SQT = 512
HALF = 1024  # sq-half width
NSQC = HALF // P  # 8 sq chunks per half
KC = D // P  # 8 contraction chunks for the input projections
BF16 = ml_dtypes.bfloat16

_CACHE = {}


def _build(s=S):
    import concourse.bass as bass
    import concourse.bacc as bacc
    import concourse.mybir as mybir
    import concourse.tile as tile
    from contextlib import ExitStack

    dt = mybir.dt
    AF = mybir.ActivationFunctionType
    DR = mybir.MatmulPerfMode.DoubleRow
    nsk = s // P  # 16 sk chunks
    nst = s // P  # 16 sq tiles (outproj)

    nc = bacc.Bacc("TRN2", target_bir_lowering=False, debug=False)
    xqT = nc.declare_dram_parameter("xqT", [D, s], dt.bfloat16, isOutput=False)
    xkT = nc.declare_dram_parameter("xkT", [D, s], dt.bfloat16, isOutput=False)
    xvT = nc.declare_dram_parameter("xvT", [D, s], dt.bfloat16, isOutput=False)
    maskT = nc.declare_dram_parameter("maskT", [s, s], dt.bfloat16, isOutput=False)
    wqT = nc.declare_dram_parameter("wqT", [D, CORE_DIMS], dt.bfloat16, isOutput=False)
    wkT = nc.declare_dram_parameter("wkT", [D, CORE_DIMS], dt.bfloat16, isOutput=False)
    wvT = nc.declare_dram_parameter("wvT", [D, CORE_DIMS], dt.bfloat16, isOutput=False)
    woT = nc.declare_dram_parameter("woT", [CORE_DIMS, D], dt.bfloat16, isOutput=False)
    out = nc.declare_dram_parameter("out", [s, D], dt.bfloat16, isOutput=True)

    with ExitStack() as ctx:
        tc = ctx.enter_context(tile.TileContext(nc))
        consts = ctx.enter_context(tc.tile_pool(name="consts", bufs=1))
        wpool = ctx.enter_context(tc.tile_pool(name="wpool", bufs=1))
        xpool = ctx.enter_context(tc.tile_pool(name="xpool", bufs=20))
        actpool = ctx.enter_context(tc.tile_pool(name="actpool", bufs=1))
        mpool = ctx.enter_context(tc.tile_pool(name="mpool", bufs=1))
        ptpool = ctx.enter_context(tc.tile_pool(name="ptpool", bufs=22))
        rpool = ctx.enter_context(tc.tile_pool(name="rpool", bufs=2))
        ctpool = ctx.enter_context(tc.tile_pool(name="ctpool", bufs=2))
        obpool = ctx.enter_context(tc.tile_pool(name="obpool", bufs=4))
        psA = ctx.enter_context(tc.tile_pool(name="psA", bufs=2, space="PSUM"))
        psPO = ctx.enter_context(tc.tile_pool(name="psPO", bufs=2, space="PSUM"))

        # warm the Exp activation table off the critical path
        warm = consts.tile([P, 1], dt.float32)
        nc.vector.memset(warm, 0.0)
        nc.scalar.activation(warm, warm, AF.Exp, scale=1.0)
        onesb = consts.tile([P, 1], dt.bfloat16)
        nc.vector.memset(onesb, 1.0)

        # --- resident weights ---
        wq_sb = wpool.tile([P, KC, CORE_DIMS], dt.bfloat16, tag="wq")
        wk_sb = wpool.tile([P, KC, CORE_DIMS], dt.bfloat16, tag="wk")
        wv_sb = wpool.tile([P, KC, CORE_DIMS], dt.bfloat16, tag="wv")
        wo_sb = wpool.tile([P, 2, D], dt.bfloat16, tag="wo")

        # --- activations ---
        qT_sb = actpool.tile([P, 2, 2, s], dt.float8e4, tag="qT")  # (st, hi/lo)
        kT_sb = actpool.tile([P, 2, s], dt.float8e4, tag="kT")
        ctxT_sb = actpool.tile([P, 2, s], dt.bfloat16, tag="ctxT")
        # V in [sk, dims] layout: (chunk, stack, hh, dh)
        vpad = actpool.tile([P, nsk, 2, 2, DH], dt.bfloat16, tag="vpad")
        mask_sb = mpool.tile([P, nsk, s], dt.bfloat16, tag="mask")

        # --- input DMAs, all on the SP sequencer in demand order ---
        def dma_w(wsb, wdr):
            nc.sync.dma_start(out=wsb, in_=wdr.rearrange("(kc p) m -> p kc m", p=P))

        def dma_x(xdr, kc, hf):
            # half-column tiles so the pool ring frees at a fine grain
            xt = xpool.tile([P, HALF], dt.bfloat16, name="x", tag="x")
            nc.sync.dma_start(
                out=xt, in_=xdr[kc * P:(kc + 1) * P, hf * HALF:(hf + 1) * HALF]
            )
            return xt

        def dma_m(c):
            nc.sync.dma_start(out=mask_sb[:, c, :], in_=maskT[c * P:(c + 1) * P, :])

        dma_w(wq_sb, wqT)
        xq = {(kc, hf): dma_x(xqT, kc, hf) for kc in range(KC) for hf in range(2)}
        dma_w(wk_sb, wkT)
        xk = {(kc, hf): dma_x(xkT, kc, hf) for kc in range(KC) for hf in range(2)}
        dma_w(wv_sb, wvT)
        xv = {(kc, hf): dma_x(xvT, kc, hf) for kc in range(KC) for hf in range(2)}
        dma_w(wo_sb, woT)
        for c in range(nsk):
            dma_m(c)

        # --- Q/K projections ---
        # stack 0 through the psA ring up front (xq/xk DMA-paced); stack 1
        # in [P,512] quarters through the psPO ring, interleaved into the
        # first scores slots so the psA scores ring never blocks on them.
        def q_cast(pp, slot, col, w):
            # hi on ScalarE, lo = pp - hi on VectorE
            with nc.allow_low_precision(reason="fp8 hi/lo scores operands"):
                dst_hi = qT_sb[:, slot, 0, col:col + w]
                nc.scalar.copy(dst_hi, pp)
                nc.vector.tensor_sub(
                    qT_sb[:, slot, 1, col:col + w], pp, dst_hi
                )

        def k_cast(pp, slot, col, w, on_act):
            with nc.allow_low_precision(reason="fp8 scores operands"):
                if on_act:
                    nc.scalar.copy(kT_sb[:, slot, col:col + w], pp)
                else:
                    nc.vector.tensor_copy(kT_sb[:, slot, col:col + w], pp)

        def proj0(xch, wsb, cast, extra_kc=None):
            pp = [
                psA.tile([P, HALF], dt.float32, name="pp", tag="psA")
                for _ in range(2)
            ]
            for kc in range(KC):
                lhsT = wsb[:, kc, 0:P]
                for grp in range(2):
                    for j in range(2):
                        nc.tensor.matmul(
                            pp[grp][:, j * SQT:(j + 1) * SQT],
                            lhsT,
                            xch[(kc, grp)][:, j * SQT:(j + 1) * SQT],
                            start=(kc == 0),
                            stop=(kc == KC - 1),
                        )
                if extra_kc is not None:
                    extra_kc(kc)
            for grp in range(2):
                cast(pp[grp], grp)

        def quarter_tiles():
            return [
                psPO.tile([P, SQT], dt.float32, name="pp", tag="po")
                for _ in range(2)
            ]

        def q1_mms(pp, qpair, kc):
            # quarters (2*qpair, 2*qpair+1) of the stack-1 Q projection
            for i in range(2):
                q = 2 * qpair + i
                nc.tensor.matmul(
                    pp[i],
                    wq_sb[:, kc, P:2 * P],
                    xq[(kc, q // 2)][:, (q % 2) * SQT:(q % 2 + 1) * SQT],
                    start=(kc == 0),
                    stop=(kc == KC - 1),
                )

        # Q stack-0 with Q stack-1 quarters 0/1 riding the same kc loop
        ppQ1a = quarter_tiles()
        proj0(
            xq,
            wq_sb,
            lambda pp, grp: q_cast(pp, 0, grp * HALF, HALF),
            extra_kc=lambda kc: q1_mms(ppQ1a, 0, kc),
        )
        for i in range(2):
            q_cast(ppQ1a[i], 1, i * SQT, SQT)
        ppQ1b = quarter_tiles()
        for kc in range(KC):
            q1_mms(ppQ1b, 1, kc)
        for i in range(2):
            q_cast(ppQ1b[i], 1, (2 + i) * SQT, SQT)

        proj0(xk, wk_sb, lambda pp, grp: k_cast(pp, 0, grp * HALF, HALF, grp == 0))

        k1pp = {}

        def k1_eighth(q, half):
            # K stack-1 quarter q, kc half `half` (fillers in early slots)
            if half == 0:
                k1pp[q] = psPO.tile([P, SQT], dt.float32, name="pp", tag="po")
            pp = k1pp[q]
            for kc in range(half * 4, half * 4 + 4):
                nc.tensor.matmul(
                    pp,
                    wk_sb[:, kc, P:2 * P],
                    xk[(kc, q // 2)][:, (q % 2) * SQT:(q % 2 + 1) * SQT],
                    start=(kc == 0),
                    stop=(kc == KC - 1),
                )
            if half == 1:
                k_cast(k1pp.pop(q), 1, q * SQT, SQT, q % 2 == 0)

        # --- pipeline state ---
        pts = {}
        po = {}
        cts = {}
        dens = {}

        def scores_exp_mask(g):
            Hh, h, c = g // 64, (g % 64) // 16, g % 16
            st, hh = h // 2, h % 2
            hp = hh * 64
            ps = psA.tile([P, HALF], dt.float32, name="ps", tag="psA")
            lhsT = kT_sb[hp:hp + 64, st:st + 1, c * P:(c + 1) * P].broadcast_to(
                [64, 2, P]
            )
            for i in range(2):
                col = Hh * HALF + i * SQT
                nc.tensor.matmul(
                    ps[:, i * SQT:(i + 1) * SQT],
                    lhsT,
                    qT_sb[hp:hp + 64, st, :, col:col + SQT],
                    start=True,
                    stop=True,
                    perf_mode=DR,
                )
            pt = ptpool.tile([P, HALF], dt.bfloat16, name="pt", tag="pt")
            nc.scalar.activation(pt, ps, AF.Exp, scale=0.125)
            nc.vector.tensor_mul(pt, pt, mask_sb[:, c, Hh * HALF:(Hh + 1) * HALF])
            pts[(Hh, h, c)] = pt

        def vproj(c):
            pv = psA.tile([P, HALF], dt.float32, name="pv", tag="psA")
            for kc in range(KC):
                nc.tensor.matmul(
                    pv[:, 0:CORE_DIMS],
                    xv[(kc, c // 8)][:, (c % 8) * P:(c % 8 + 1) * P],
                    wv_sb[:, kc, :],
                    start=(kc == 0),
                    stop=(kc == KC - 1),
                )
            for st in range(2):
                nc.vector.tensor_copy(
                    vpad[:, c, st, :, :], pv[:, st * P:(st + 1) * P]
                )

        def attnv(a):
            Hh, h, c = a // 64, (a % 64) // 16, a % 16
            st, hh = h // 2, h % 2
            if c == 0:
                po[(Hh, h)] = psPO.tile(
                    [P, NSQC, DH], dt.float32, name="po", tag="po"
                )
                dens[(Hh, h)] = psPO.tile(
                    [P, NSQC], dt.float32, name="den", tag="den", bufs=2
                )
            p = po[(Hh, h)]
            dn = dens[(Hh, h)]
            pt = pts.pop((Hh, h, c))
            for sqc in range(NSQC):
                lhsT = pt[:, sqc * P:(sqc + 1) * P]
                # one accumulation group per 2KB PSUM bank: start/stop only
                # on the first/last write of each tile
                nc.tensor.matmul(
                    p[:, sqc, :],
                    lhsT,
                    vpad[:, c, st, hh, :],
                    start=(c == 0 and sqc == 0),
                    stop=(c == nsk - 1 and sqc == NSQC - 1),
                )
                nc.tensor.matmul(
                    dn[:, sqc:sqc + 1],
                    lhsT,
                    onesb,
                    start=(c == 0 and sqc == 0),
                    stop=(c == nsk - 1 and sqc == NSQC - 1),
                )

        def norm(Hh, h):
            st, hh = h // 2, h % 2
            p = po.pop((Hh, h))
            if hh == 0:
                cts[(Hh, st)] = ctpool.tile(
                    [P, NSQC, 2, DH], dt.bfloat16, name="ct", tag="ct"
                )
            ct = cts[(Hh, st)]
            dn = dens.pop((Hh, h))
            rden = rpool.tile([P, NSQC], dt.float32, name="rden", tag="r")
            nc.vector.reciprocal(rden, dn)
            for sqc in range(NSQC):
                nc.vector.tensor_scalar_mul(
                    ct[:, sqc, hh, :], p[:, sqc, :], rden[:, sqc:sqc + 1]
                )
            if hh == 1:
                # restore [dims, sq] layout: one [128,128] DMA-transpose
                # per sq-chunk
                ct2 = cts.pop((Hh, st))
                for sqc in range(NSQC):
                    nc.sync.dma_start(
                        out=ctxT_sb[:, st, Hh * HALF + sqc * P:
                                    Hh * HALF + (sqc + 1) * P],
                        in_=ct2[:, sqc, :, :],
                        transpose=True,
                    )

        def outproj(stile, use_act=False):
            # psum rides the po ring, keeping the scores (psA) ring clean
            ob = obpool.tile([P, D], dt.bfloat16, name="ob", tag="ob")
            for dcol in range(2):
                pp = psPO.tile([P, SQT], dt.float32, name="ppo", tag="po")
                for st in range(2):
                    nc.tensor.matmul(
                        pp,
                        ctxT_sb[:, st, stile * P:(stile + 1) * P],
                        wo_sb[:, st, dcol * SQT:(dcol + 1) * SQT],
                        start=(st == 0),
                        stop=(st == 1),
                    )
                if dcol == 0 and use_act:
                    nc.scalar.copy(ob[:, dcol * SQT:(dcol + 1) * SQT], pp)
                else:
                    nc.vector.tensor_copy(
                        ob[:, dcol * SQT:(dcol + 1) * SQT], pp
                    )
            nc.sync.dma_start(out=out[stile * P:(stile + 1) * P, :], in_=ob)

        # --- schedule: scores stream g=0..127; attnV trails by a lag ---
        NG = 128
        sched = {}

        def add(slot, fn):
            sched.setdefault(slot, []).append(fn)

        # K stack-1 eighth-chunks occupy the first scores slots
        for q in range(4):
            for half in range(2):
                add(2 * q + half, (lambda q=q, half=half: k1_eighth(q, half)))

        for a in range(NG):
            Hh, h, c = a // 64, (a % 64) // 16, a % 16
            # head 0's attnV follows the vproj fillers, which go every OTHER
            # slot so the scores psA ring keeps its full double-buffer depth
            if a < 16:
                slot = 12 + 2 * c
                add(slot, (lambda c=c: vproj(c)))
                add(slot, (lambda a=a: attnv(a)))
            else:
                lag = 8 if Hh == 0 else 2
                slot = a + lag
                add(slot, (lambda a=a: attnv(a)))
            if c == 15:
                add(slot + 1, (lambda Hh=Hh, h=h: norm(Hh, h)))
        # outproj of sq-half 0 interleaved into half 1's stream
        for i, slot in enumerate(range(80, 112, 4)):
            add(slot, (lambda i=i: outproj(i)))

        max_slot = max(sched)
        for g in range(max_slot + 1):
            if g < NG:
                scores_exp_mask(g)
            for fn in sched.get(g, []):
                fn()
        # tail: remaining outproj tiles
        for i in range(8, nst):
            outproj(i, use_act=True)

    nc.compile()
    return nc


def _shard_inputs(query, key, value, mask, wq, wk, wv, wo):
    query = np.asarray(query, dtype=np.float32)
    key = np.asarray(key, dtype=np.float32)
    value = np.asarray(value, dtype=np.float32)
    mask = np.asarray(mask)
    wq = np.asarray(wq, dtype=np.float32)
    wk = np.asarray(wk, dtype=np.float32)
    wv = np.asarray(wv, dtype=np.float32)
    wo = np.asarray(wo, dtype=np.float32)

    xT = []
    mT = []
    for b in range(B):
        xT.append(
            tuple(
                np.ascontiguousarray(a[b].T).astype(BF16)
                for a in (query, key, value)
            )
        )
        mT.append(np.ascontiguousarray(mask[b].T).astype(BF16))

    in_maps = []
    for c in range(N_CORES):
        b, g = c // 4, c % 4
        hsel = slice(g * CORE_DIMS, (g + 1) * CORE_DIMS)
        in_maps.append(
            {
                "xqT": xT[b][0],
                "xkT": xT[b][1],
                "xvT": xT[b][2],
                "maskT": mT[b],
                "wqT": np.ascontiguousarray(wq[hsel].T).astype(BF16),
                "wkT": np.ascontiguousarray(wk[hsel].T).astype(BF16),
                "wvT": np.ascontiguousarray(wv[hsel].T).astype(BF16),
                "woT": np.ascontiguousarray(wo[:, hsel].T).astype(BF16),
            }
        )
    return in_maps


LAST_RESULTS = None  # BassKernelResults of the most recent kernel() call


def kernel(query, key, value, mask, wq, wk, wv, wo):
    global LAST_RESULTS
    from concourse import bass_utils

    if "nc" not in _CACHE:
        _CACHE["nc"] = _build()
    nc = _CACHE["nc"]

    in_maps = _shard_inputs(query, key, value, mask, wq, wk, wv, wo)
    res = bass_utils.run_bass_kernel_spmd(nc, in_maps, core_ids=list(range(N_CORES)))
    LAST_RESULTS = res

    outp = np.empty((B, S, D), dtype=np.float32)
    for b in range(B):
        acc = res.results[4 * b]["out"].astype(np.float32)
        for g in range(1, 4):
            acc = acc + res.results[4 * b + g]["out"].astype(np.float32)
        outp[b] = acc
    return outp


# revision 43
# speedup vs baseline: 1.4772x; 1.0184x over previous
"""Multi-head attention (B=2, S=2048, D=1024, H=16) on 8 TRN2 NeuronCores.

Sharding: tensor-parallel over heads x data-parallel over batch.
Core c handles batch b = c//4 and heads 4*(c%4) .. 4*(c%4)+3 (256 of the
1024 projected dims).  wq/wk/wv are split column-wise, wo row-wise; each
core emits a [S, D] bf16 partial of the output projection and the host
sums the 4 partials per batch.

Per-core kernel:
  1. Q^T/K^T projections in bf16; outputs cast to fp8e4m3.  Q is stored
     as a hi/lo fp8 split (q ~= q_hi + q_lo at ~bf16 precision).
  2. Scores for head h, sk-chunk c as ONE fp8 DoubleRow matmul: the
     stationary k8 tile is duplicated via a stride-0 AP, the moving pair
     is (q_hi, q_lo), so PSUM accumulates k8*(q_hi+q_lo) = k8*q at
     0.5 cycles/row.  exp on ScalarE (scale=1/8, no max subtraction),
     mask multiply on VectorE.
  3. V projected directly to [sk, dims] layout (lhsT = x chunk).
  4. attnV transposed: lhsT = P^T sq-chunk (M=128 full PE width), rhs =
     V columns (N=64) accumulating ctx^T' = [sq, dh] in 1-bank PSUM
     tiles; a parallel N=1 ones-matmul accumulates the softmax
     denominators per sq partition.
  5. Normalization per head: fp32 reciprocal of the denominator column,
     per-partition tensor_scalar multiply into a paired [sq, (hh, dh)]
     bf16 staging tile; an SBUF->SBUF DMA-transpose of each [128, 128]
     block restores the ctxT [dims, sq] layout for the output projection.
  6. Output projection accumulates both stacks in PSUM and stores one
     bf16 [128, D] tile per sq-chunk; the first half is interleaved into
     the second half's attention stream, the rest forms the tail.
All DMAs (loads, transposes, stores) issue from the SP sequencer.
"""

import sys

if "/opt/trn_rl_repo" not in sys.path:
    sys.path.insert(0, "/opt/trn_rl_repo")

import numpy as np
import ml_dtypes

B = 2
S = 2048
D = 1024
H = 16
DH = 64
P = 128
N_CORES = 8
HEADS_PER_CORE = 4
CORE_DIMS = HEADS_PER_CORE * DH  # 256
SQT = 512
HALF = 1024  # sq-half width
NSQC = HALF // P  # 8 sq chunks per half
KC = D // P  # 8 contraction chunks for the input projections
BF16 = ml_dtypes.bfloat16

_CACHE = {}


def _build(s=S):
    import concourse.bass as bass
    import concourse.bacc as bacc
    import concourse.mybir as mybir
    import concourse.tile as tile
    from contextlib import ExitStack

    dt = mybir.dt
    AF = mybir.ActivationFunctionType
    DR = mybir.MatmulPerfMode.DoubleRow
    nsk = s // P  # 16 sk chunks
    nst = s // P  # 16 sq tiles (outproj)

    nc = bacc.Bacc("TRN2", target_bir_lowering=False, debug=False)
    xqT = nc.declare_dram_parameter("xqT", [D, s], dt.bfloat16, isOutput=False)
    xkT = nc.declare_dram_parameter("xkT", [D, s], dt.bfloat16, isOutput=False)
    xvT = nc.declare_dram_parameter("xvT", [D, s], dt.bfloat16, isOutput=False)
    maskT = nc.declare_dram_parameter("maskT", [s, s], dt.bfloat16, isOutput=False)
    wqT = nc.declare_dram_parameter("wqT", [D, CORE_DIMS], dt.bfloat16, isOutput=False)
    wkT = nc.declare_dram_parameter("wkT", [D, CORE_DIMS], dt.bfloat16, isOutput=False)
    wvT = nc.declare_dram_parameter("wvT", [D, CORE_DIMS], dt.bfloat16, isOutput=False)
    woT = nc.declare_dram_parameter("woT", [CORE_DIMS, D], dt.bfloat16, isOutput=False)
    out = nc.declare_dram_parameter("out", [s, D], dt.bfloat16, isOutput=True)

    with ExitStack() as ctx:
        tc = ctx.enter_context(tile.TileContext(nc))
        consts = ctx.enter_context(tc.tile_pool(name="consts", bufs=1))
        wpool = ctx.enter_context(tc.tile_pool(name="wpool", bufs=1))
        xpool = ctx.enter_context(tc.tile_pool(name="xpool", bufs=20))
        actpool = ctx.enter_context(tc.tile_pool(name="actpool", bufs=1))
        mpool = ctx.enter_context(tc.tile_pool(name="mpool", bufs=1))
        ptpool = ctx.enter_context(tc.tile_pool(name="ptpool", bufs=22))
        rpool = ctx.enter_context(tc.tile_pool(name="rpool", bufs=2))
        ctpool = ctx.enter_context(tc.tile_pool(name="ctpool", bufs=2))
        obpool = ctx.enter_context(tc.tile_pool(name="obpool", bufs=4))
        psA = ctx.enter_context(tc.tile_pool(name="psA", bufs=2, space="PSUM"))
        psPO = ctx.enter_context(tc.tile_pool(name="psPO", bufs=2, space="PSUM"))

        # warm the Exp activation table off the critical path
        warm = consts.tile([P, 1], dt.float32)
        nc.vector.memset(warm, 0.0)
        nc.scalar.activation(warm, warm, AF.Exp, scale=1.0)
        onesb = consts.tile([P, 1], dt.bfloat16)
        nc.vector.memset(onesb, 1.0)

        # --- resident weights ---
        wq_sb = wpool.tile([P, KC, CORE_DIMS], dt.bfloat16, tag="wq")
        wk_sb = wpool.tile([P, KC, CORE_DIMS], dt.bfloat16, tag="wk")
        wv_sb = wpool.tile([P, KC, CORE_DIMS], dt.bfloat16, tag="wv")
        wo_sb = wpool.tile([P, 2, D], dt.bfloat16, tag="wo")

        # --- activations ---
        qT_sb = actpool.tile([P, 2, 2, s], dt.float8e4, tag="qT")  # (st, hi/lo)
        kT_sb = actpool.tile([P, 2, s], dt.float8e4, tag="kT")
        ctxT_sb = actpool.tile([P, 2, s], dt.bfloat16, tag="ctxT")
        # V in [sk, dims] layout: (chunk, stack, hh, dh)
        vpad = actpool.tile([P, nsk, 2, 2, DH], dt.bfloat16, tag="vpad")
        mask_sb = mpool.tile([P, nsk, s], dt.bfloat16, tag="mask")

        # --- input DMAs, all on the SP sequencer in demand order ---
        def dma_w(wsb, wdr):
            nc.sync.dma_start(out=wsb, in_=wdr.rearrange("(kc p) m -> p kc m", p=P))

        def dma_x(xdr, kc, hf):
            # half-column tiles so the pool ring frees at a fine grain
            xt = xpool.tile([P, HALF], dt.bfloat16, name="x", tag="x")
            nc.sync.dma_start(
                out=xt, in_=xdr[kc * P:(kc + 1) * P, hf * HALF:(hf + 1) * HALF]
            )
            return xt

        def dma_m(c):
            nc.sync.dma_start(out=mask_sb[:, c, :], in_=maskT[c * P:(c + 1) * P, :])

        dma_w(wq_sb, wqT)
        xq = {(kc, hf): dma_x(xqT, kc, hf) for kc in range(KC) for hf in range(2)}
        dma_w(wk_sb, wkT)
        xk = {(kc, hf): dma_x(xkT, kc, hf) for kc in range(KC) for hf in range(2)}
        dma_w(wv_sb, wvT)
        xv = {(kc, hf): dma_x(xvT, kc, hf) for kc in range(KC) for hf in range(2)}
        dma_w(wo_sb, woT)
        for c in range(nsk):
            dma_m(c)

        # --- Q/K projections ---
        # stack 0 through the psA ring up front (xq/xk DMA-paced); stack 1
        # in [P,512] quarters through the psPO ring, interleaved into the
        # first scores slots so the psA scores ring never blocks on them.
        def q_cast(pp, slot, col, w):
            # hi on ScalarE, lo = pp - hi on VectorE
            with nc.allow_low_precision(reason="fp8 hi/lo scores operands"):
                dst_hi = qT_sb[:, slot, 0, col:col + w]
                nc.scalar.copy(dst_hi, pp)
                nc.vector.tensor_sub(
                    qT_sb[:, slot, 1, col:col + w], pp, dst_hi
                )

        def k_cast(pp, slot, col, w, on_act):
            with nc.allow_low_precision(reason="fp8 scores operands"):
                if on_act:
                    nc.scalar.copy(kT_sb[:, slot, col:col + w], pp)
                else:
                    nc.vector.tensor_copy(kT_sb[:, slot, col:col + w], pp)

        def proj0(xch, wsb, cast, extra_kc=None):
            pp = [
                psA.tile([P, HALF], dt.float32, name="pp", tag="psA")
                for _ in range(2)
            ]
            for kc in range(KC):
                lhsT = wsb[:, kc, 0:P]
                for grp in range(2):
                    for j in range(2):
                        nc.tensor.matmul(
                            pp[grp][:, j * SQT:(j + 1) * SQT],
                            lhsT,
                            xch[(kc, grp)][:, j * SQT:(j + 1) * SQT],
                            start=(kc == 0),
                            stop=(kc == KC - 1),
                        )
                if extra_kc is not None:
                    extra_kc(kc)
            for grp in range(2):
                cast(pp[grp], grp)

        def quarter_tiles():
            return [
                psPO.tile([P, SQT], dt.float32, name="pp", tag="po")
                for _ in range(2)
            ]

        def q1_mms(pp, qpair, kc):
            # quarters (2*qpair, 2*qpair+1) of the stack-1 Q projection
            for i in range(2):
                q = 2 * qpair + i
                nc.tensor.matmul(
                    pp[i],
                    wq_sb[:, kc, P:2 * P],
                    xq[(kc, q // 2)][:, (q % 2) * SQT:(q % 2 + 1) * SQT],
                    start=(kc == 0),
                    stop=(kc == KC - 1),
                )

        # Q stack-0 with Q stack-1 quarters 0/1 riding the same kc loop
        ppQ1a = quarter_tiles()
        proj0(
            xq,
            wq_sb,
            lambda pp, grp: q_cast(pp, 0, grp * HALF, HALF),
            extra_kc=lambda kc: q1_mms(ppQ1a, 0, kc),
        )
        for i in range(2):
            q_cast(ppQ1a[i], 1, i * SQT, SQT)
        ppQ1b = quarter_tiles()
        for kc in range(KC):
            q1_mms(ppQ1b, 1, kc)
        for i in range(2):
            q_cast(ppQ1b[i], 1, (2 + i) * SQT, SQT)

        proj0(xk, wk_sb, lambda pp, grp: k_cast(pp, 0, grp * HALF, HALF, grp == 0))

        k1pp = {}

        def k1_eighth(q, half):
            # K stack-1 quarter q, kc half `half` (fillers in early slots)
            if half == 0:
                k1pp[q] = psPO.tile([P, SQT], dt.float32, name="pp", tag="po")
            pp = k1pp[q]
            for kc in range(half * 4, half * 4 + 4):
                nc.tensor.matmul(
                    pp,
                    wk_sb[:, kc, P:2 * P],
                    xk[(kc, q // 2)][:, (q % 2) * SQT:(q % 2 + 1) * SQT],
                    start=(kc == 0),
                    stop=(kc == KC - 1),
                )
            if half == 1:
                k_cast(k1pp.pop(q), 1, q * SQT, SQT, q % 2 == 0)

        # --- pipeline state ---
        pts = {}
        po = {}
        cts = {}
        dens = {}

        def scores_exp_mask(g):
            Hh, h, c = g // 64, (g % 64) // 16, g % 16
            st, hh = h // 2, h % 2
            hp = hh * 64
            ps = psA.tile([P, HALF], dt.float32, name="ps", tag="psA")
            lhsT = kT_sb[hp:hp + 64, st:st + 1, c * P:(c + 1) * P].broadcast_to(
                [64, 2, P]
            )
            for i in range(2):
                col = Hh * HALF + i * SQT
                nc.tensor.matmul(
                    ps[:, i * SQT:(i + 1) * SQT],
                    lhsT,
                    qT_sb[hp:hp + 64, st, :, col:col + SQT],
                    start=True,
                    stop=True,
                    perf_mode=DR,
                )
            pt = ptpool.tile([P, HALF], dt.bfloat16, name="pt", tag="pt")
            nc.scalar.activation(pt, ps, AF.Exp, scale=0.125)
            pts[(Hh, h, c)] = pt

        def mask_mul(a):
            # deferred near the attnV slot so a not-yet-arrived mask DMA
            # never head-of-line-blocks the DVE queue
            Hh, h, c = a // 64, (a % 64) // 16, a % 16
            pt = pts[(Hh, h, c)]
            nc.vector.tensor_mul(pt, pt, mask_sb[:, c, Hh * HALF:(Hh + 1) * HALF])

        def vproj(c):
            pv = psA.tile([P, HALF], dt.float32, name="pv", tag="psA")
            for kc in range(KC):
                nc.tensor.matmul(
                    pv[:, 0:CORE_DIMS],
                    xv[(kc, c // 8)][:, (c % 8) * P:(c % 8 + 1) * P],
                    wv_sb[:, kc, :],
                    start=(kc == 0),
                    stop=(kc == KC - 1),
                )
            for st in range(2):
                nc.vector.tensor_copy(
                    vpad[:, c, st, :, :], pv[:, st * P:(st + 1) * P]
                )

        def attnv(a):
            Hh, h, c = a // 64, (a % 64) // 16, a % 16
            st, hh = h // 2, h % 2
            if c == 0:
                po[(Hh, h)] = psPO.tile(
                    [P, NSQC, DH], dt.float32, name="po", tag="po"
                )
                dens[(Hh, h)] = psPO.tile(
                    [P, NSQC], dt.float32, name="den", tag="den", bufs=2
                )
            p = po[(Hh, h)]
            dn = dens[(Hh, h)]
            pt = pts.pop((Hh, h, c))
            for sqc in range(NSQC):
                lhsT = pt[:, sqc * P:(sqc + 1) * P]
                # one accumulation group per 2KB PSUM bank: start/stop only
                # on the first/last write of each tile
                nc.tensor.matmul(
                    p[:, sqc, :],
                    lhsT,
                    vpad[:, c, st, hh, :],
                    start=(c == 0 and sqc == 0),
                    stop=(c == nsk - 1 and sqc == NSQC - 1),
                )
                nc.tensor.matmul(
                    dn[:, sqc:sqc + 1],
                    lhsT,
                    onesb,
                    start=(c == 0 and sqc == 0),
                    stop=(c == nsk - 1 and sqc == NSQC - 1),
                )

        def norm(Hh, h):
            st, hh = h // 2, h % 2
            p = po.pop((Hh, h))
            if hh == 0:
                cts[(Hh, st)] = ctpool.tile(
                    [P, NSQC, 2, DH], dt.bfloat16, name="ct", tag="ct"
                )
            ct = cts[(Hh, st)]
            dn = dens.pop((Hh, h))
            rden = rpool.tile([P, NSQC], dt.float32, name="rden", tag="r")
            nc.vector.reciprocal(rden, dn)
            for sqc in range(NSQC):
                nc.vector.tensor_scalar_mul(
                    ct[:, sqc, hh, :], p[:, sqc, :], rden[:, sqc:sqc + 1]
                )
            if hh == 1:
                # restore [dims, sq] layout: one [128,128] DMA-transpose
                # per sq-chunk
                ct2 = cts.pop((Hh, st))
                for sqc in range(NSQC):
                    nc.sync.dma_start(
                        out=ctxT_sb[:, st, Hh * HALF + sqc * P:
                                    Hh * HALF + (sqc + 1) * P],
                        in_=ct2[:, sqc, :, :],
                        transpose=True,
                    )

        def outproj(stile, use_act=False):
            # psum rides the po ring, keeping the scores (psA) ring clean
            ob = obpool.tile([P, D], dt.bfloat16, name="ob", tag="ob")
            for dcol in range(2):
                pp = psPO.tile([P, SQT], dt.float32, name="ppo", tag="po")
                for st in range(2):
                    nc.tensor.matmul(
                        pp,
                        ctxT_sb[:, st, stile * P:(stile + 1) * P],
                        wo_sb[:, st, dcol * SQT:(dcol + 1) * SQT],
                        start=(st == 0),
                        stop=(st == 1),
                    )
                if dcol == 0 and use_act:
                    nc.scalar.copy(ob[:, dcol * SQT:(dcol + 1) * SQT], pp)
                else:
                    nc.vector.tensor_copy(
                        ob[:, dcol * SQT:(dcol + 1) * SQT], pp
                    )
            nc.sync.dma_start(out=out[stile * P:(stile + 1) * P, :], in_=ob)

        # --- schedule: scores stream g=0..127; attnV trails by a lag ---
        NG = 128
        sched = {}

        def add(slot, fn):
            sched.setdefault(slot, []).append(fn)

        # K stack-1 eighth-chunks occupy the first scores slots
        for q in range(4):
            for half in range(2):
                add(2 * q + half, (lambda q=q, half=half: k1_eighth(q, half)))

        for a in range(NG):
            Hh, h, c = a // 64, (a % 64) // 16, a % 16
            # head 0's attnV follows the vproj fillers, which go every OTHER
            # slot so the scores psA ring keeps its full double-buffer depth
            if a < 16:
                slot = 12 + 2 * c
                add(slot - 1, (lambda a=a: mask_mul(a)))
                add(slot, (lambda c=c: vproj(c)))
                add(slot, (lambda a=a: attnv(a)))
            else:
                lag = 8 if Hh == 0 else 2
                slot = a + lag
                if a < 32:
                    # head 1 must not overtake the 2-slot-paced vproj stream
                    slot = max(slot, 13 + 2 * c)
                add(slot - 1, (lambda a=a: mask_mul(a)))
                add(slot, (lambda a=a: attnv(a)))
            if c == 15:
                # h1's norm must follow h0's (it reuses h0's ct tile and
                # issues the pair's transposes); h0's norm sits at slot 43
                # due to the stretched vproj schedule
                add(max(slot + 1, 44) if (Hh, h) == (0, 1) else slot + 1,
                    (lambda Hh=Hh, h=h: norm(Hh, h)))
        # outproj of sq-half 0 interleaved into half 1's stream
        for i, slot in enumerate(range(80, 112, 4)):
            add(slot, (lambda i=i: outproj(i)))

        max_slot = max(sched)
        for g in range(max_slot + 1):
            if g < NG:
                scores_exp_mask(g)
            for fn in sched.get(g, []):
                fn()
        # tail: remaining outproj tiles
        for i in range(8, nst):
            outproj(i, use_act=True)

    nc.compile()
    return nc


def _shard_inputs(query, key, value, mask, wq, wk, wv, wo):
    query = np.asarray(query, dtype=np.float32)
    key = np.asarray(key, dtype=np.float32)
    value = np.asarray(value, dtype=np.float32)
    mask = np.asarray(mask)
    wq = np.asarray(wq, dtype=np.float32)
    wk = np.asarray(wk, dtype=np.float32)
    wv = np.asarray(wv, dtype=np.float32)
    wo = np.asarray(wo, dtype=np.float32)

    xT = []
    mT = []
    for b in range(B):
        xT.append(
            tuple(
                np.ascontiguousarray(a[b].T).astype(BF16)
                for a in (query, key, value)
            )
        )
        mT.append(np.ascontiguousarray(mask[b].T).astype(BF16))

    in_maps = []
    for c in range(N_CORES):
        b, g = c // 4, c % 4
        hsel = slice(g * CORE_DIMS, (g + 1) * CORE_DIMS)
        in_maps.append(
            {
                "xqT": xT[b][0],
                "xkT": xT[b][1],
                "xvT": xT[b][2],
                "maskT": mT[b],
                "wqT": np.ascontiguousarray(wq[hsel].T).astype(BF16),
                "wkT": np.ascontiguousarray(wk[hsel].T).astype(BF16),
                "wvT": np.ascontiguousarray(wv[hsel].T).astype(BF16),
                "woT": np.ascontiguousarray(wo[:, hsel].T).astype(BF16),
            }
        )
    return in_maps


LAST_RESULTS = None  # BassKernelResults of the most recent kernel() call


def kernel(query, key, value, mask, wq, wk, wv, wo):
    global LAST_RESULTS
    from concourse import bass_utils

    if "nc" not in _CACHE:
        _CACHE["nc"] = _build()
    nc = _CACHE["nc"]

    in_maps = _shard_inputs(query, key, value, mask, wq, wk, wv, wo)
    res = bass_utils.run_bass_kernel_spmd(nc, in_maps, core_ids=list(range(N_CORES)))
    LAST_RESULTS = res

    outp = np.empty((B, S, D), dtype=np.float32)
    for b in range(B):
        acc = res.results[4 * b]["out"].astype(np.float32)
        for g in range(1, 4):
            acc = acc + res.results[4 * b + g]["out"].astype(np.float32)
        outp[b] = acc
    return outp
